# revision 35
# baseline (speedup 1.0000x reference)
"""Additive attention kernel for Trainium2 (8 NeuronCores, SPMD).

Reference computation (B=4, L=1024, D=256, U=128):
    q = X @ W1                                   [B,L,U]
    k = X @ W2                                   [B,L,U]
    g = tanh(q[:,:,None,:] + k[:,None,:,:] + b1) [B,L,L,U]
    s = sigmoid(g @ W3 + b2)                     [B,L,L]
    out = s @ X                                  [B,L,D]

Sharding: 8 cores = (batch b, query-half h).  Each core handles 512 queries
against all 1024 keys of its batch.

Algorithm: the L*L*U tanh tensor is never materialized.  tanh is
approximated by a 4-term Fourier sine series  tanh(x) ~ sum_r c_r sin(w_r x)
(fit on |x|<=13, gaussian-weighted; end-to-end rel err ~3.5e-3), and
sin(w(q+k)) = sin(wq)cos(wk) + cos(wq)sin(wk) turns the score computation
into a plain matmul with contraction dim 2R*U = 8*128:

    z[q,k] = sum_u W3_u tanh(q_u + k_u + b1_u)
           ~ sum_i Fq_i[u,q] . Fk_i[u,k]        (8 accumulating PE matmuls)

The HW Sin spline is only valid on [-pi, pi]; arguments for the three
higher frequencies are range-reduced with the magic-constant trick
(f = u - round(u), round via +-1.5*2^23, fp16 tensors -- the DVE ALU
computes in fp32 so the fp32 magic still rounds correctly) on DVE; cos
comes from a +0.25-shifted copy of the reduction.  The lowest frequency
is in range directly (cos via +pi/2 bias).

sigmoid(z+b2) = 0.5*tanh(0.5*z + 0.5*b2) + 0.5 keeps everything in one ACT
table set (sin+tanh, silu_and_others).  The +0.5 term becomes a rank-1
colsum(X) matmul accumulated into the output PSUM group and the global 0.5
factor is applied in the final PSUM->SBUF output copies.

All matmuls run in fp16 (1 cycle/row): factors and tanh scores are written
fp16 by ACT directly, X/W get one fp16 cast each (ACT identity for X,
DVE for W).  PE order is software-pipelined (scores of block kb+1 issue
before the outputs of block kb so the PE never waits on tanh), dummy
activations precede the DMA triggers on the scalar queue so the activation
table loads overlap the input DMA, and the gpsimd DMA queue (which sits
behind the ~7us kernel preamble) only carries late-needed tensors.
"""

import numpy as np

B, L, D, U = 4, 1024, 256, 128
QH = L // 2          # queries per core
N_CORES = 8
R = 4

# Fourier fit of tanh on [-13,13], gaussian-weighted (sigma^2=2.67)
OMEGA = [0.2177, 0.7144, 1.48488, 2.49552]
COEF = [1.226339, 0.458469, 0.174421, 0.045835]
MAGIC = float(1.5 * 2 ** 23)
TWO_PI = float(2 * np.pi)

_CACHE = {}
LAST_RESULTS = None


def _build_program():
    import concourse.bass as bass
    import concourse.bacc as bacc
    import concourse.mybir as mybir
    import concourse.tile as tile
    from concourse import masks

    f32 = mybir.dt.float32
    f16 = mybir.dt.float16
    AF = mybir.ActivationFunctionType
    ALU = mybir.AluOpType

    nc = bacc.Bacc(
        "TRN2",
        target_bir_lowering=False,
        debug=False,
        enable_asserts=False,
        num_devices=N_CORES,
    )

    Xb = nc.dram_tensor("Xb", [L, D], f32, kind="ExternalInput")
    Xq = nc.dram_tensor("Xq", [QH, D], f32, kind="ExternalInput")
    W1 = nc.dram_tensor("W1", [D, U], f32, kind="ExternalInput")
    W2 = nc.dram_tensor("W2", [D, U], f32, kind="ExternalInput")
    W3 = nc.dram_tensor("W3v", [U, 1], f32, kind="ExternalInput")
    b1 = nc.dram_tensor("b1", [U, 1], f32, kind="ExternalInput")
    b2 = nc.dram_tensor("b2", [1, 1], f32, kind="ExternalInput")
    out = nc.dram_tensor("out", [QH, D], f32, kind="ExternalOutput")

    NLB = L // 128   # 8 key blocks
    NQB = QH // 128  # 4 query blocks
    NDB = D // 128   # 2 d blocks
    NI = 2 * R       # factor pairs

    with tile.TileContext(nc) as tc:
        with (
            tc.tile_pool(name="const", bufs=1) as cp,
            tc.tile_pool(name="score", bufs=2) as scp,
            tc.tile_pool(name="outs", bufs=2) as outp,
            tc.tile_pool(name="pre_ps", bufs=2, space="PSUM") as prepsum,
            tc.tile_pool(name="score_ps", bufs=2, space="PSUM") as scorepsum,
            tc.tile_pool(name="out_ps", bufs=1, space="PSUM") as outpsum,
        ):
            # ---- constants first: the dummy activations must precede the
            # scalar-queue DMA triggers so the table load runs at t=0 ----
            dum = cp.tile([1, 1], f32)
            nc.vector.memset(dum[:], 0.0)
            dumo = cp.tile([1, 2], f32)
            nc.scalar.activation(dumo[:, 0:1], dum[:], AF.Sin, bias=0.0)
            nc.scalar.activation(dumo[:, 1:2], dum[:], AF.Tanh, bias=0.0)

            # ---- input DMA; queue choice = arrival priority.  sync and
            # scalar start immediately; gpsimd sits behind the preamble ----
            Xqs = cp.tile([128, NQB, D], f32)
            nc.sync.dma_start(
                Xqs[:, 0:NQB // 2, :],
                Xq[0:QH // 2].rearrange("(qb p) d -> p qb d", p=128))
            nc.scalar.dma_start(
                Xqs[:, NQB // 2:, :],
                Xq[QH // 2:QH].rearrange("(qb p) d -> p qb d", p=128))
            W1s = cp.tile([128, NDB, U], f32)
            nc.sync.dma_start(W1s[:], W1[:].rearrange("(db p) u -> p db u", p=128))
            Xbs = cp.tile([128, NLB, D], f32)
            nc.sync.dma_start(
                Xbs[:, 0:2, :],
                Xb[0:256].rearrange("(kb p) d -> p kb d", p=128))
            nc.scalar.dma_start(
                Xbs[:, 4:6, :],
                Xb[512:768].rearrange("(kb p) d -> p kb d", p=128))
            nc.sync.dma_start(
                Xbs[:, 2:4, :],
                Xb[256:512].rearrange("(kb p) d -> p kb d", p=128))
            b1s = cp.tile([128, 1], f32)
            nc.sync.dma_start(b1s[:], b1[:])
            W2s = cp.tile([128, NDB, U], f32)
            nc.sync.dma_start(W2s[:], W2[:].rearrange("(db p) u -> p db u", p=128))
            nc.gpsimd.dma_start(
                Xbs[:, 6:8, :],
                Xb[768:1024].rearrange("(kb p) d -> p kb d", p=128))
            W3s = cp.tile([128, 1], f32)
            nc.gpsimd.dma_start(W3s[:], W3[:])
            b2s = cp.tile([1, 1], f32)
            nc.gpsimd.dma_start(b2s[:], b2[:])

            ident = cp.tile([128, 128], f32)
            masks.make_identity(nc, ident[:])
            identh = cp.tile([128, 128], f16)
            masks.make_identity(nc, identh[:])
            halfpi = cp.tile([128, 1], f32)
            nc.vector.memset(halfpi[:], float(np.pi / 2))
            ones_col = cp.tile([128, 1], f16)
            nc.vector.memset(ones_col[:], 1.0)
            ones_row = cp.tile([1, 128], f16)
            nc.vector.memset(ones_row[:], 1.0)
            ones_row32 = cp.tile([1, 128], f32)
            nc.vector.memset(ones_row32[:], 1.0)

            # fp16 weights (tiny DVE casts)
            W1h = cp.tile([128, NDB, U], f16)
            nc.vector.tensor_copy(W1h[:], W1s[:])
            W2h = cp.tile([128, NDB, U], f16)
            nc.vector.tensor_copy(W2h[:], W2s[:])

            # ---- Xq transposes (PE fp32; DVE copies cast psum -> fp16) ----
            xqT = cp.tile([128, NDB, QH], f16)
            for qb in range(NQB):
                for db in range(NDB):
                    tp = prepsum.tile([128, 128], f32, tag="pre", name="tp")
                    nc.tensor.transpose(
                        tp[:], Xqs[:, qb, db * 128:(db + 1) * 128], ident[:])
                    nc.vector.tensor_copy(
                        xqT[:, db, qb * 128:(qb + 1) * 128], tp[:])

            # ---- qT[u,q] = W1^T Xq^T (stays in PSUM; ACT/DVE read it) ----
            qpre = prepsum.tile([128, QH], f32, tag="qkpre", name="qpre",
                                bufs=2)
            for db in range(NDB):
                nc.tensor.matmul(qpre[:], W1h[:, db, :], xqT[:, db, :],
                                 start=(db == 0), stop=(db == NDB - 1))

            # fp16 X for the k transposes and colsum/output matmuls
            # (first half early on ACT, second half after its DMA)
            Xbh = cp.tile([128, NLB, D], f16)
            nc.scalar.activation(Xbh[:, 0:NLB // 2, :], Xbs[:, 0:NLB // 2, :],
                                 AF.Identity, bias=0.0)

            # ---- q-side r0 factors from a fp16 copy (frees the qpre
            # PSUM bank so kpre h2 is not blocked by late q readers) ----
            Fq = cp.tile([128, NI, QH], f16)
            qT16 = cp.tile([128, QH], f16)
            nc.vector.tensor_copy(qT16[:], qpre[:])
            nc.scalar.activation(Fq[:, 0, :], qT16[:], AF.Sin,
                                 bias=0.0, scale=OMEGA[0])
            nc.scalar.activation(Fq[:, 1, :], qT16[:], AF.Sin,
                                 bias=halfpi[:], scale=OMEGA[0])

            # second fp16 X half (must precede the kb4-7 transposes)
            nc.scalar.activation(Xbh[:, NLB // 2:, :], Xbs[:, NLB // 2:, :],
                                 AF.Identity, bias=0.0)

            # k-side r0 bias tiles (fold b1) + per-frequency b1 offsets
            kb_s = cp.tile([128, 1], f32)
            nc.vector.tensor_scalar_mul(kb_s[:], b1s[:], OMEGA[0])
            kb_c = cp.tile([128, 1], f32)
            nc.vector.tensor_scalar(kb_c[:], b1s[:], OMEGA[0],
                                    float(np.pi / 2),
                                    op0=ALU.mult, op1=ALU.add)
            b1u = cp.tile([128, R - 1], f32)
            for j in range(R - 1):
                nc.vector.tensor_scalar_mul(b1u[:, j:j + 1], b1s[:],
                                            OMEGA[j + 1] / TWO_PI)

            # ---- Xb transposes (fp16 from the ACT-cast Xbh) ----
            xbT = cp.tile([128, NDB, L], f16)
            for kb in range(NLB):
                for db in range(NDB):
                    tp = prepsum.tile([128, 128], f16, tag="pre", name="tp")
                    nc.tensor.transpose(
                        tp[:], Xbh[:, kb, db * 128:(db + 1) * 128], identh[:])
                    nc.vector.tensor_copy(
                        xbT[:, db, kb * 128:(kb + 1) * 128], tp[:])


            # ---- k-side: kpre halves -> fp16 reductions + beta rows ----
            Fk = cp.tile([128, NI, L], f16)
            uk = cp.tile([128, R - 1, 2, L], f16)
            ruk = cp.tile([128, R - 1, 2, L], f16)
            fk = cp.tile([128, R - 1, 2, L], f16)
            for lh in range(2):
                kp = prepsum.tile([128, QH], f32, tag="qkpre",
                                  name=f"kpre{lh}", bufs=2)
                for db in range(NDB):
                    nc.tensor.matmul(
                        kp[:], W2h[:, db, :],
                        xbT[:, db, lh * QH:(lh + 1) * QH],
                        start=(db == 0), stop=(db == NDB - 1))
                s = slice(lh * QH, (lh + 1) * QH)
                # r0 factors, phi flipped on the k side (0=cos, 1=sin)
                nc.scalar.activation(Fk[:, 0, s], kp[:], AF.Sin,
                                     bias=kb_c[:], scale=OMEGA[0])
                nc.scalar.activation(Fk[:, 1, s], kp[:], AF.Sin,
                                     bias=kb_s[:], scale=OMEGA[0])
                # reductions: slot 1 = sin (u), slot 0 = cos (u + 0.25)
                for j in range(R - 1):
                    nc.vector.tensor_scalar(uk[:, j, 1, s], kp[:],
                                            OMEGA[j + 1] / TWO_PI,
                                            b1u[:, j:j + 1],
                                            op0=ALU.mult, op1=ALU.add)
                nc.vector.tensor_scalar_add(uk[:, :, 0, s], uk[:, :, 1, s],
                                            0.25)
                nc.vector.tensor_scalar(ruk[:, :, :, s], uk[:, :, :, s],
                                        MAGIC, MAGIC,
                                        op0=ALU.add, op1=ALU.subtract)
                nc.vector.tensor_tensor(fk[:, :, :, s], uk[:, :, :, s],
                                        ruk[:, :, :, s], op=ALU.subtract)
                nc.scalar.activation(Fk[:, 2:NI, s], fk[:, :, :, s], AF.Sin,
                                     bias=0.0, scale=TWO_PI)
                if lh == 0:
                    # q reduction (fp16 all-SBUF, fast DVE) + merged q sine
                    # + coefficients, overlapping the k h2 chain
                    uq = cp.tile([128, R - 1, 2, QH], f16)
                    for j in range(R - 1):
                        nc.vector.tensor_scalar_mul(uq[:, j, 0, :], qT16[:],
                                                    OMEGA[j + 1] / TWO_PI)
                    nc.vector.tensor_scalar_add(uq[:, :, 1, :],
                                                uq[:, :, 0, :], 0.25)
                    ruq = cp.tile([128, R - 1, 2, QH], f16)
                    nc.vector.tensor_scalar(ruq[:], uq[:], MAGIC, MAGIC,
                                            op0=ALU.add, op1=ALU.subtract)
                    fq = cp.tile([128, R - 1, 2, QH], f16)
                    nc.vector.tensor_tensor(fq[:], uq[:], ruq[:],
                                            op=ALU.subtract)
                    nc.scalar.activation(Fq[:, 2:NI, :], fq[:], AF.Sin,
                                         bias=0.0, scale=TWO_PI)
                    Acoef = cp.tile([128, R], f32)
                    for r in range(R):
                        nc.vector.tensor_scalar_mul(Acoef[:, r:r + 1],
                                                    W3s[:], COEF[r])
                    for i in range(NI):
                        nc.vector.tensor_scalar(
                            Fq[:, i, :], Fq[:, i, :],
                            Acoef[:, i // 2:i // 2 + 1], None, op0=ALU.mult)


            # ---- colsum(X) doubled + 0.5*b2 broadcast ----
            csp = prepsum.tile([1, D], f32, tag="pre", name="csp")
            for kb in range(NLB):
                nc.tensor.matmul(csp[:], ones_col[:], Xbh[:, kb, :],
                                 start=(kb == 0), stop=(kb == NLB - 1))
            csh2 = cp.tile([1, 2, D], f16)
            nc.vector.tensor_copy(csh2[:, 0, :], csp[:])
            nc.vector.tensor_copy(csh2[:, 1, :], csp[:])
            tpb = prepsum.tile([128, 1], f32, tag="pre", name="tpb")
            nc.tensor.matmul(tpb[:], ones_row32[:], b2s[:])
            b2h = cp.tile([128, 1], f32)
            nc.vector.tensor_scalar_mul(b2h[:], tpb[:], 0.5)

            # ---- output accumulators: two banks, two query blocks each;
            # the rank-1 colsum term starts each bank's group ----
            po = [outpsum.tile([128, 2 * D], f32, tag=f"po{h}", name=f"po{h}")
                  for h in range(2)]
            for h in range(2):
                nc.tensor.matmul(po[h][:], ones_row[:], csh2[:],
                                 start=True, stop=False, skip_group_check=True)

            # ---- main loop; outputs lag one block so PE never waits ----
            pending = None

            def emit_outs(kb, scT):
                for qs in range(NQB):
                    nc.tensor.matmul(
                        po[qs // 2][:, (qs % 2) * D:(qs % 2 + 1) * D],
                        scT[:, qs * 128:(qs + 1) * 128],
                        Xbh[:, kb, :],
                        start=False,
                        stop=(kb == NLB - 1 and qs % 2 == 1),
                        skip_group_check=True)

            for kb in range(NLB):
                scpre = scorepsum.tile([128, QH], f32, name="scpre")
                for i in range(NI):
                    nc.tensor.matmul(
                        scpre[:], Fk[:, i, kb * 128:(kb + 1) * 128],
                        Fq[:, i, :],
                        start=(i == 0), stop=(i == NI - 1))

                scT = scp.tile([128, QH], f16, tag="scT", name="scT")
                nc.scalar.activation(scT[:], scpre[:], AF.Tanh,
                                     bias=b2h[:], scale=0.5)
                if pending is not None:
                    emit_outs(*pending)
                pending = (kb, scT)
            emit_outs(*pending)

            # ---- write out: 0.5 * po  (the sigmoid half-factor) ----
            for qs in range(NQB):
                ot = outp.tile([128, D], f32, tag="ot", name="ot")
                nc.vector.tensor_scalar_mul(
                    ot[:], po[qs // 2][:, (qs % 2) * D:(qs % 2 + 1) * D], 0.5)
                eng = nc.sync if qs % 2 == 0 else nc.scalar
                eng.dma_start(out[qs * 128:(qs + 1) * 128, :], ot[:])

    # The act-table chooser picks the first set containing each function,
    # which ping-pongs between exp_and_others (tanh) and trig_and_small
    # (sin).  silu_and_others genuinely contains both sin and tanh;
    # restrict membership (indices unchanged, so emitted set ids stay
    # valid) so one load covers the whole kernel.
    from concourse.hw_specs import get_activation_tables
    tabs = get_activation_tables(nc.m.arch)
    for name, fns in tabs.items():
        if name != "silu_and_others":
            fns.discard(AF.Sin)
            fns.discard(AF.Tanh)

    nc.compile()
    return nc


def _get_nc():
    if "nc" not in _CACHE:
        _CACHE["nc"] = _build_program()
    return _CACHE["nc"]


def kernel(X, W1, W2, W3, bias1, bias2, trace=False):
    global LAST_RESULTS
    from concourse.bass_utils import run_bass_kernel_spmd

    X = np.ascontiguousarray(np.asarray(X, dtype=np.float32))
    W1 = np.ascontiguousarray(np.asarray(W1, dtype=np.float32))
    W2 = np.ascontiguousarray(np.asarray(W2, dtype=np.float32))
    W3 = np.ascontiguousarray(np.asarray(W3, dtype=np.float32))
    b1 = np.ascontiguousarray(np.asarray(bias1, dtype=np.float32).reshape(U, 1))
    b2 = np.ascontiguousarray(np.asarray(bias2, dtype=np.float32).reshape(1, 1))

    nc = _get_nc()
    in_maps = []
    for c in range(N_CORES):
        b, h = c // 2, c % 2
        in_maps.append({
            "Xb": X[b],
            "Xq": np.ascontiguousarray(X[b, h * QH:(h + 1) * QH]),
            "W1": W1,
            "W2": W2,
            "W3v": W3,
            "b1": b1,
            "b2": b2,
        })

    res = run_bass_kernel_spmd(nc, in_maps, core_ids=list(range(N_CORES)),
                               trace=trace)
    LAST_RESULTS = res

    out = np.empty((B, L, D), dtype=np.float32)
    for c in range(N_CORES):
        b, h = c // 2, c % 2
        out[b, h * QH:(h + 1) * QH] = res.results[c]["out"]
    return out


# revision 36
# speedup vs baseline: 1.0557x; 1.0557x over previous
"""Additive attention kernel for Trainium2 (8 NeuronCores, SPMD).

Reference computation (B=4, L=1024, D=256, U=128):
    q = X @ W1                                   [B,L,U]
    k = X @ W2                                   [B,L,U]
    g = tanh(q[:,:,None,:] + k[:,None,:,:] + b1) [B,L,L,U]
    s = sigmoid(g @ W3 + b2)                     [B,L,L]
    out = s @ X                                  [B,L,D]

Sharding: 8 cores = (batch b, query-half h).  Each core handles 512 queries
against all 1024 keys of its batch.

Algorithm: the L*L*U tanh tensor is never materialized.  tanh is
approximated by a 4-term Fourier sine series  tanh(x) ~ sum_r c_r sin(w_r x)
(fit on |x|<=13, gaussian-weighted; end-to-end rel err ~3.5e-3), and
sin(w(q+k)) = sin(wq)cos(wk) + cos(wq)sin(wk) turns the score computation
into a plain matmul with contraction dim 2R*U = 8*128:

    z[q,k] = sum_u W3_u tanh(q_u + k_u + b1_u)
           ~ sum_i Fq_i[u,q] . Fk_i[u,k]        (8 accumulating PE matmuls)

The HW Sin spline is only valid on [-pi, pi]; arguments for the three
higher frequencies are range-reduced with the magic-constant trick
(f = u - round(u), round via +-1.5*2^23, fp16 tensors -- the DVE ALU
computes in fp32 so the fp32 magic still rounds correctly) on DVE; cos
comes from a +0.25-shifted copy of the reduction.  The lowest frequency
is in range directly (cos via +pi/2 bias).

sigmoid(z+b2) = 0.5*tanh(0.5*z + 0.5*b2) + 0.5 keeps everything in one ACT
table set (sin+tanh, silu_and_others).  The +0.5 term becomes a rank-1
colsum(X) matmul accumulated into the output PSUM group and the global 0.5
factor is applied in the final PSUM->SBUF output copies.

All matmuls run in fp16 (1 cycle/row): factors and tanh scores are written
fp16 by ACT directly, X/W get one fp16 cast each (ACT identity for X,
DVE for W).  PE order is software-pipelined (scores of block kb+1 issue
before the outputs of block kb so the PE never waits on tanh), dummy
activations precede the DMA triggers on the scalar queue so the activation
table loads overlap the input DMA, and the gpsimd DMA queue (which sits
behind the ~7us kernel preamble) only carries late-needed tensors.
"""

import numpy as np

B, L, D, U = 4, 1024, 256, 128
QH = L // 2          # queries per core
N_CORES = 8
R = 4

# Fourier fit of tanh on [-13,13], gaussian-weighted (sigma^2=2.67)
OMEGA = [0.2177, 0.7144, 1.48488, 2.49552]
COEF = [1.226339, 0.458469, 0.174421, 0.045835]
MAGIC = float(1.5 * 2 ** 23)
TWO_PI = float(2 * np.pi)

_CACHE = {}
LAST_RESULTS = None


def _build_program():
    import concourse.bass as bass
    import concourse.bacc as bacc
    import concourse.mybir as mybir
    import concourse.tile as tile
    from concourse import masks

    f32 = mybir.dt.float32
    f16 = mybir.dt.float16
    AF = mybir.ActivationFunctionType
    ALU = mybir.AluOpType

    nc = bacc.Bacc(
        "TRN2",
        target_bir_lowering=False,
        debug=False,
        enable_asserts=False,
        num_devices=N_CORES,
    )

    Xb = nc.dram_tensor("Xb", [L, D], f32, kind="ExternalInput")
    Xq = nc.dram_tensor("Xq", [QH, D], f32, kind="ExternalInput")
    W1 = nc.dram_tensor("W1", [D, U], f32, kind="ExternalInput")
    W2 = nc.dram_tensor("W2", [D, U], f32, kind="ExternalInput")
    W3 = nc.dram_tensor("W3v", [U, 1], f32, kind="ExternalInput")
    b1 = nc.dram_tensor("b1", [U, 1], f32, kind="ExternalInput")
    b2 = nc.dram_tensor("b2", [1, 1], f32, kind="ExternalInput")
    out = nc.dram_tensor("out", [QH, D], f32, kind="ExternalOutput")

    NLB = L // 128   # 8 key blocks
    NQB = QH // 128  # 4 query blocks
    NDB = D // 128   # 2 d blocks
    NI = 2 * R       # factor pairs

    with tile.TileContext(nc) as tc:
        with (
            tc.tile_pool(name="const", bufs=1) as cp,
            tc.tile_pool(name="score", bufs=3) as scp,
            tc.tile_pool(name="outs", bufs=2) as outp,
            tc.tile_pool(name="pre_ps", bufs=2, space="PSUM") as prepsum,
            tc.tile_pool(name="score_ps", bufs=2, space="PSUM") as scorepsum,
            tc.tile_pool(name="out_ps", bufs=1, space="PSUM") as outpsum,
        ):
            # ---- constants first: the dummy activations must precede the
            # scalar-queue DMA triggers so the table load runs at t=0 ----
            dum = cp.tile([1, 1], f32)
            nc.vector.memset(dum[:], 0.0)
            dumo = cp.tile([1, 2], f32)
            nc.scalar.activation(dumo[:, 0:1], dum[:], AF.Sin, bias=0.0)
            nc.scalar.activation(dumo[:, 1:2], dum[:], AF.Tanh, bias=0.0)

            # ---- input DMA; queue choice = arrival priority.  sync and
            # scalar start immediately; gpsimd sits behind the preamble ----
            Xqs = cp.tile([128, NQB, D], f32)
            nc.sync.dma_start(
                Xqs[:, 0:NQB // 2, :],
                Xq[0:QH // 2].rearrange("(qb p) d -> p qb d", p=128))
            nc.scalar.dma_start(
                Xqs[:, NQB // 2:, :],
                Xq[QH // 2:QH].rearrange("(qb p) d -> p qb d", p=128))
            W1s = cp.tile([128, NDB, U], f32)
            nc.sync.dma_start(W1s[:], W1[:].rearrange("(db p) u -> p db u", p=128))
            Xbs = cp.tile([128, NLB, D], f32)
            nc.sync.dma_start(
                Xbs[:, 0:2, :],
                Xb[0:256].rearrange("(kb p) d -> p kb d", p=128))
            nc.scalar.dma_start(
                Xbs[:, 4:6, :],
                Xb[512:768].rearrange("(kb p) d -> p kb d", p=128))
            nc.sync.dma_start(
                Xbs[:, 2:4, :],
                Xb[256:512].rearrange("(kb p) d -> p kb d", p=128))
            b1s = cp.tile([128, 1], f32)
            nc.sync.dma_start(b1s[:], b1[:])
            W2s = cp.tile([128, NDB, U], f32)
            nc.sync.dma_start(W2s[:], W2[:].rearrange("(db p) u -> p db u", p=128))
            nc.gpsimd.dma_start(
                Xbs[:, 6:8, :],
                Xb[768:1024].rearrange("(kb p) d -> p kb d", p=128))
            W3s = cp.tile([128, 1], f32)
            nc.gpsimd.dma_start(W3s[:], W3[:])
            b2s = cp.tile([1, 1], f32)
            nc.gpsimd.dma_start(b2s[:], b2[:])

            ident = cp.tile([128, 128], f32)
            masks.make_identity(nc, ident[:])
            identh = cp.tile([128, 128], f16)
            masks.make_identity(nc, identh[:])
            halfpi = cp.tile([128, 1], f32)
            nc.vector.memset(halfpi[:], float(np.pi / 2))
            ones_col = cp.tile([128, 1], f16)
            nc.vector.memset(ones_col[:], 1.0)
            ones_row = cp.tile([1, 128], f16)
            nc.vector.memset(ones_row[:], 1.0)
            ones_row32 = cp.tile([1, 128], f32)
            nc.vector.memset(ones_row32[:], 1.0)

            # fp16 weights (tiny DVE casts)
            W1h = cp.tile([128, NDB, U], f16)
            nc.vector.tensor_copy(W1h[:], W1s[:])
            W2h = cp.tile([128, NDB, U], f16)
            nc.vector.tensor_copy(W2h[:], W2s[:])

            # ---- Xq transposes (PE fp32; DVE copies cast psum -> fp16) ----
            xqT = cp.tile([128, NDB, QH], f16)
            for qb in range(NQB):
                for db in range(NDB):
                    tp = prepsum.tile([128, 128], f32, tag="pre", name="tp")
                    nc.tensor.transpose(
                        tp[:], Xqs[:, qb, db * 128:(db + 1) * 128], ident[:])
                    nc.vector.tensor_copy(
                        xqT[:, db, qb * 128:(qb + 1) * 128], tp[:])

            # ---- qT[u,q] = W1^T Xq^T (stays in PSUM; ACT/DVE read it) ----
            qpre = prepsum.tile([128, QH], f32, tag="qkpre", name="qpre",
                                bufs=2)
            for db in range(NDB):
                nc.tensor.matmul(qpre[:], W1h[:, db, :], xqT[:, db, :],
                                 start=(db == 0), stop=(db == NDB - 1))

            # fp16 X for the k transposes and colsum/output matmuls
            # (first half early on ACT, second half after its DMA)
            Xbh = cp.tile([128, NLB, D], f16)
            nc.scalar.activation(Xbh[:, 0:NLB // 2, :], Xbs[:, 0:NLB // 2, :],
                                 AF.Identity, bias=0.0)

            # ---- q-side r0 factors from a fp16 copy (frees the qpre
            # PSUM bank so kpre h2 is not blocked by late q readers) ----
            Fq = cp.tile([128, NI, QH], f16)
            qT16 = cp.tile([128, QH], f16)
            nc.vector.tensor_copy(qT16[:], qpre[:])
            nc.scalar.activation(Fq[:, 0, :], qT16[:], AF.Sin,
                                 bias=0.0, scale=OMEGA[0])
            nc.scalar.activation(Fq[:, 1, :], qT16[:], AF.Sin,
                                 bias=halfpi[:], scale=OMEGA[0])

            # second fp16 X half (must precede the kb4-7 transposes)
            nc.scalar.activation(Xbh[:, NLB // 2:, :], Xbs[:, NLB // 2:, :],
                                 AF.Identity, bias=0.0)

            # k-side r0 bias tiles (fold b1) + per-frequency b1 offsets
            kb_s = cp.tile([128, 1], f32)
            nc.vector.tensor_scalar_mul(kb_s[:], b1s[:], OMEGA[0])
            kb_c = cp.tile([128, 1], f32)
            nc.vector.tensor_scalar(kb_c[:], b1s[:], OMEGA[0],
                                    float(np.pi / 2),
                                    op0=ALU.mult, op1=ALU.add)
            b1u = cp.tile([128, R - 1], f32)
            for j in range(R - 1):
                nc.vector.tensor_scalar_mul(b1u[:, j:j + 1], b1s[:],
                                            OMEGA[j + 1] / TWO_PI)

            # ---- Xb transposes (fp16 from the ACT-cast Xbh) ----
            xbT = cp.tile([128, NDB, L], f16)
            for kb in range(NLB):
                for db in range(NDB):
                    tp = prepsum.tile([128, 128], f16, tag="pre", name="tp")
                    nc.tensor.transpose(
                        tp[:], Xbh[:, kb, db * 128:(db + 1) * 128], identh[:])
                    nc.vector.tensor_copy(
                        xbT[:, db, kb * 128:(kb + 1) * 128], tp[:])


            # ---- k-side: kpre halves -> fp16 reductions + beta rows ----
            Fk = cp.tile([128, NI, L], f16)
            uk = cp.tile([128, R - 1, 2, L], f16)
            ruk = cp.tile([128, R - 1, 2, L], f16)
            fk = cp.tile([128, R - 1, 2, L], f16)
            for lh in range(2):
                kp = prepsum.tile([128, QH], f32, tag="qkpre",
                                  name=f"kpre{lh}", bufs=2)
                for db in range(NDB):
                    nc.tensor.matmul(
                        kp[:], W2h[:, db, :],
                        xbT[:, db, lh * QH:(lh + 1) * QH],
                        start=(db == 0), stop=(db == NDB - 1))
                s = slice(lh * QH, (lh + 1) * QH)
                # r0 factors, phi flipped on the k side (0=cos, 1=sin)
                nc.scalar.activation(Fk[:, 0, s], kp[:], AF.Sin,
                                     bias=kb_c[:], scale=OMEGA[0])
                nc.scalar.activation(Fk[:, 1, s], kp[:], AF.Sin,
                                     bias=kb_s[:], scale=OMEGA[0])
                # reductions: slot 1 = sin (u), slot 0 = cos (u + 0.25)
                for j in range(R - 1):
                    nc.vector.tensor_scalar(uk[:, j, 1, s], kp[:],
                                            OMEGA[j + 1] / TWO_PI,
                                            b1u[:, j:j + 1],
                                            op0=ALU.mult, op1=ALU.add)
                nc.vector.tensor_scalar_add(uk[:, :, 0, s], uk[:, :, 1, s],
                                            0.25)
                nc.vector.tensor_scalar(ruk[:, :, :, s], uk[:, :, :, s],
                                        MAGIC, MAGIC,
                                        op0=ALU.add, op1=ALU.subtract)
                nc.vector.tensor_tensor(fk[:, :, :, s], uk[:, :, :, s],
                                        ruk[:, :, :, s], op=ALU.subtract)
                nc.scalar.activation(Fk[:, 2:NI, s], fk[:, :, :, s], AF.Sin,
                                     bias=0.0, scale=TWO_PI)
                if lh == 0:
                    # q reduction (fp16 all-SBUF, fast DVE) + merged q sine
                    # + coefficients, overlapping the k h2 chain
                    uq = cp.tile([128, R - 1, 2, QH], f16)
                    for j in range(R - 1):
                        nc.vector.tensor_scalar_mul(uq[:, j, 0, :], qT16[:],
                                                    OMEGA[j + 1] / TWO_PI)
                    nc.vector.tensor_scalar_add(uq[:, :, 1, :],
                                                uq[:, :, 0, :], 0.25)
                    ruq = cp.tile([128, R - 1, 2, QH], f16)
                    nc.vector.tensor_scalar(ruq[:], uq[:], MAGIC, MAGIC,
                                            op0=ALU.add, op1=ALU.subtract)
                    fq = cp.tile([128, R - 1, 2, QH], f16)
                    nc.vector.tensor_tensor(fq[:], uq[:], ruq[:],
                                            op=ALU.subtract)
                    nc.scalar.activation(Fq[:, 2:NI, :], fq[:], AF.Sin,
                                         bias=0.0, scale=TWO_PI)
                    Acoef = cp.tile([128, R], f32)
                    for r in range(R):
                        nc.vector.tensor_scalar_mul(Acoef[:, r:r + 1],
                                                    W3s[:], COEF[r])
                    for i in range(NI):
                        nc.vector.tensor_scalar(
                            Fq[:, i, :], Fq[:, i, :],
                            Acoef[:, i // 2:i // 2 + 1], None, op0=ALU.mult)


            # ---- colsum(X) doubled + 0.5*b2 broadcast ----
            csp = prepsum.tile([1, D], f32, tag="pre", name="csp")
            for kb in range(NLB):
                nc.tensor.matmul(csp[:], ones_col[:], Xbh[:, kb, :],
                                 start=(kb == 0), stop=(kb == NLB - 1))
            csh2 = cp.tile([1, 2, D], f16)
            nc.vector.tensor_copy(csh2[:, 0, :], csp[:])
            nc.vector.tensor_copy(csh2[:, 1, :], csp[:])
            tpb = prepsum.tile([128, 1], f32, tag="pre", name="tpb")
            nc.tensor.matmul(tpb[:], ones_row32[:], b2s[:])
            b2h = cp.tile([128, 1], f32)
            nc.vector.tensor_scalar_mul(b2h[:], tpb[:], 0.5)

            # ---- output accumulators: two banks, two query blocks each;
            # the rank-1 colsum term starts each bank's group ----
            po = [outpsum.tile([128, 2 * D], f32, tag=f"po{h}", name=f"po{h}")
                  for h in range(2)]
            for h in range(2):
                nc.tensor.matmul(po[h][:], ones_row[:], csh2[:],
                                 start=True, stop=False, skip_group_check=True)

            # ---- main loop; outputs lag two blocks so PE never waits
            # on tanh even transiently ----
            pending = []

            def emit_outs(kb, scT):
                for qs in range(NQB):
                    nc.tensor.matmul(
                        po[qs // 2][:, (qs % 2) * D:(qs % 2 + 1) * D],
                        scT[:, qs * 128:(qs + 1) * 128],
                        Xbh[:, kb, :],
                        start=False,
                        stop=(kb == NLB - 1 and qs % 2 == 1),
                        skip_group_check=True)

            for kb in range(NLB):
                scpre = scorepsum.tile([128, QH], f32, name="scpre")
                for i in range(NI):
                    nc.tensor.matmul(
                        scpre[:], Fk[:, i, kb * 128:(kb + 1) * 128],
                        Fq[:, i, :],
                        start=(i == 0), stop=(i == NI - 1))

                scT = scp.tile([128, QH], f16, tag="scT", name="scT")
                nc.scalar.activation(scT[:], scpre[:], AF.Tanh,
                                     bias=b2h[:], scale=0.5)
                pending.append((kb, scT))
                if len(pending) > 2:
                    emit_outs(*pending.pop(0))
            for p in pending:
                emit_outs(*p)

            # ---- write out: 0.5 * po  (the sigmoid half-factor) ----
            for qs in range(NQB):
                ot = outp.tile([128, D], f32, tag="ot", name="ot")
                nc.vector.tensor_scalar_mul(
                    ot[:], po[qs // 2][:, (qs % 2) * D:(qs % 2 + 1) * D], 0.5)
                eng = nc.sync if qs % 2 == 0 else nc.scalar
                eng.dma_start(out[qs * 128:(qs + 1) * 128, :], ot[:])

    # The act-table chooser picks the first set containing each function,
    # which ping-pongs between exp_and_others (tanh) and trig_and_small
    # (sin).  silu_and_others genuinely contains both sin and tanh;
    # restrict membership (indices unchanged, so emitted set ids stay
    # valid) so one load covers the whole kernel.
    from concourse.hw_specs import get_activation_tables
    tabs = get_activation_tables(nc.m.arch)
    for name, fns in tabs.items():
        if name != "silu_and_others":
            fns.discard(AF.Sin)
            fns.discard(AF.Tanh)

    nc.compile()
    return nc


def _get_nc():
    if "nc" not in _CACHE:
        _CACHE["nc"] = _build_program()
    return _CACHE["nc"]


def kernel(X, W1, W2, W3, bias1, bias2, trace=False):
    global LAST_RESULTS
    from concourse.bass_utils import run_bass_kernel_spmd

    X = np.ascontiguousarray(np.asarray(X, dtype=np.float32))
    W1 = np.ascontiguousarray(np.asarray(W1, dtype=np.float32))
    W2 = np.ascontiguousarray(np.asarray(W2, dtype=np.float32))
    W3 = np.ascontiguousarray(np.asarray(W3, dtype=np.float32))
    b1 = np.ascontiguousarray(np.asarray(bias1, dtype=np.float32).reshape(U, 1))
    b2 = np.ascontiguousarray(np.asarray(bias2, dtype=np.float32).reshape(1, 1))

    nc = _get_nc()
    in_maps = []
    for c in range(N_CORES):
        b, h = c // 2, c % 2
        in_maps.append({
            "Xb": X[b],
            "Xq": np.ascontiguousarray(X[b, h * QH:(h + 1) * QH]),
            "W1": W1,
            "W2": W2,
            "W3v": W3,
            "b1": b1,
            "b2": b2,
        })

    res = run_bass_kernel_spmd(nc, in_maps, core_ids=list(range(N_CORES)),
                               trace=trace)
    LAST_RESULTS = res

    out = np.empty((B, L, D), dtype=np.float32)
    for c in range(N_CORES):
        b, h = c // 2, c % 2
        out[b, h * QH:(h + 1) * QH] = res.results[c]["out"]
    return out
